# revision 35
# baseline (speedup 1.0000x reference)
r"""Trainium2 Bass kernel for causal average pooling (downsampling).

Reference op: out[b, i, d] = mean(x[b, :(i+1)*4, d]) over the time axis,
for x of shape (8, 8192, 512) f32 -> out (8, 2048, 512) f32.

Strategy
--------
Data-parallel over batch: one batch per NeuronCore (8 cores), no
cross-core communication.

HBM-bound (~358 GB/s/core), so both HBM legs are bf16 (tolerance 2e-2,
this pipeline measures ~5e-3): the host casts x to bf16 (halves loads)
and the output is stored bf16, upcast on host (halves stores).
Per-core traffic 21 -> 10.5 MB.

Compute is a 3-level pairwise pooling + prefix scan + two scaled output
phases, laid out so every DVE op runs in the packed bf16 mode (2 cols/
cycle, 0.5 ns/col - strided or fp32 ops are 2-3x slower): the host
groups each piece's time columns mod 8 (pure permutation) so the pools
are contiguous block adds, merged across sub-blocks with 3D APs:

    [a|b|c|d] = [x0|x2|x4|x6] + [x1|x3|x5|x7]     # one TT op (L1)
    [s4e|s4o] = [a|c] + [b|d]                     # one TT op (L2)
    cs   = pair-scan(s4e, s4o)        # 8 x-cols/step, fp32 state
    tmp  = s4e + cs[k-1]              # TT (cs col 0 = 0)
    oe   = tmp * 1/(4(2k+1))          # \ one merged TT over [p,2,n8]
    oo   = cs  * 1/(8(k+1))           # / (tmp lives in cs's row 0)

Scans chain across a tile's pieces via initial=AP (measured free), so
cs is globally cumulative: no carry columns and tmp's k-1 shift is one
offset AP read, correct across piece boundaries.  Outputs land
phase-split (ot=[evens|odds]; host re-interleaves) and the recip row
[even|odd|ones] is broadcast to all partitions by a ones[1,128].T @ row
matmul on the idle PE (banks 0,2,1,3 so low slices of both halves
arrive first), PSUM -> bf16 rt via ACT copies.

Tile 3 rides the SWDGE accumulate path instead: 4 semaphore-ordered
dma_starts (first plain, rest accum_op=add via the CCE ALU in the SDMA
datapath) land its s4 = [s4e|s4o] with ZERO DVE work.  CCE accumulate
is slow (~100 GB/s measured vs ~350 plain) but runs on its own queue
concurrently with the plain loads, so one tile's worth trickles in for
free while DVE pools the other three - pure DVE offload.  Two chains,
emitted hop-major as a pair so each hop's wait on its predecessor is
already satisfied; the tile ends with a RAW piece (no chain latency in
the tail).  Tile 0 ramps up with two small RAW pieces so DVE starts
within ~1 us.

All raw Bass; every cross-engine wait is a standalone wait_ge.
"""

import sys

if "/opt/trn_rl_repo" not in sys.path:
    sys.path.insert(0, "/opt/trn_rl_repo")

import ml_dtypes
import numpy as np

import concourse.bass as bass
import concourse.mybir as mybir
from concourse.bass_utils import run_bass_kernel_spmd

P = 128           # SBUF partitions
SF = 4            # pooling factor
B, L, D = 8, 8192, 512
N_CORES = 8
BF16 = ml_dtypes.bfloat16

HOPS = {"raw": 1, "acc4": 4}


def _plan(length=L, n_ct=4):
    """Per-tile pieces: (xs, xe, mode).  'raw' = plain SP-ring load +
    DVE pools; 'acc4' = 4-hop SWDGE accumulate chain (no DVE pooling)."""
    q = length // 8
    t0 = [
        (0, q, "raw"),
        (q, 2 * q, "raw"),
        (2 * q, 4 * q, "raw"),
        (4 * q, length, "raw"),
    ]
    mid = [(0, length // 2, "raw"), (length // 2, length, "raw")]
    tn = [
        (0, 3 * q, "acc4"),
        (3 * q, 5 * q, "acc4"),
        (5 * q, 7 * q, "acc4"),
        (7 * q, length, "raw"),
    ]
    return [t0] + [mid] * (n_ct - 2) + [tn]


def build_bass(d=D, length=L):
    out_len = length // SF
    half = out_len // 2            # phase block width (1024)
    n_ct = d // P
    assert d % P == 0 and length % 64 == 0

    nc = bass.Bass()
    xT = nc.dram_tensor("xT", [d, length], mybir.dt.bfloat16, kind="ExternalInput")
    recip = nc.dram_tensor(
        "recip", [1, out_len + P], mybir.dt.float32, kind="ExternalInput"
    )
    outT = nc.dram_tensor(
        "outT", [d, 2, half], mybir.dt.bfloat16, kind="ExternalOutput"
    )

    plan = _plan(length, n_ct)
    raws_all = [
        (ct, p) for ct in range(n_ct)
        for p, (_x, _e, m) in enumerate(plan[ct]) if m == "raw"
    ]
    max_raw = max((xe - xs) for t in plan for (xs, xe, m) in t if m == "raw")

    # DVE op bookkeeping (s_cmp incremented by every DVE op).
    cmp_val = 0
    pool_val = {}   # (ct,p) -> last RAW pool op index (xtr free)
    tmp_val = {}    # (ct,p) -> tmp op index (last s4 reader)
    out_val = {}    # (ct,p) -> merged out-mul op index

    # The far tail's RAW piece: its load and pools are hoisted into the
    # cold-start window (DVE would otherwise idle waiting early loads).
    hoisted = [
        (ct, p) for (ct, p) in raws_all
        if p >= 1 and plan[ct][0][2] != "raw"
    ]
    raws = (
        [raws_all[0]] + hoisted
        + [k for k in raws_all[1:] if k not in hoisted]
    )
    raw_idx = {k: i for i, k in enumerate(raws)}

    def _ops():
        yield ("pools", 0, 0)
        for ct, p in hoisted:
            yield ("pools", ct, p)
        for ct in range(n_ct):
            for p, (xs, xe, mode) in enumerate(plan[ct]):
                if mode == "raw" and (ct, p) != (0, 0) and (ct, p) not in hoisted:
                    yield ("pools", ct, p)
                yield ("scan", ct, p)
                yield ("tmp", ct, p)
                yield ("omul", ct, p)

    for kind, ct, p in _ops():
        cmp_val += 2 if kind == "pools" else 1
        if kind == "pools":
            pool_val[(ct, p)] = cmp_val
        elif kind == "tmp":
            tmp_val[(ct, p)] = cmp_val
        elif kind == "omul":
            out_val[(ct, p)] = cmp_val

    n_xbuf = 4   # one s4 buffer per tile: accumulate chains start at t=0

    with bass.ExitStack() as stack:
        en = stack.enter_context
        s4s = [
            en(nc.sbuf_tensor(f"s4_{i}", [P, 2, half], mybir.dt.bfloat16))
            for i in range(n_xbuf)
        ]
        xtrs = [
            en(nc.sbuf_tensor(f"xtr_{i}", [P, max_raw], mybir.dt.bfloat16))
            for i in range(4)
        ]
        s2r = en(nc.sbuf_tensor([P, max_raw // 2], mybir.dt.bfloat16))
        # cs buffers: row 1 = scan states (global), row 0 = tmp, so the
        # merged out-mul reads [p, 2, n8] in one op.  No leading zero col:
        # tmp = cs[k] - s4o[k] (same value as s4e[k] + cs[k-1]), so every
        # packed bf16 access sits at an even element offset - odd-offset
        # packed reads corrupt tail lanes (measured as NaNs).
        css = [
            en(nc.sbuf_tensor(f"cs_{i}", [P, 2, half], mybir.dt.bfloat16))
            for i in range(2)
        ]
        rrow = en(nc.sbuf_tensor([1, out_len + P], mybir.dt.float32))
        ones = en(nc.sbuf_tensor([1, P], mybir.dt.float32))
        scr = en(nc.sbuf_tensor([1, 1], mybir.dt.float32))
        rps = en(nc.psum_tensor([P, out_len], mybir.dt.float32))
        rt = en(nc.sbuf_tensor([P, 2, half], mybir.dt.bfloat16))
        ot = en(nc.sbuf_tensor([P, n_ct, 2, half], mybir.dt.bfloat16))
        s_rrow = en(nc.semaphore("s_rrow"))
        s_ones = en(nc.semaphore("s_ones"))
        s_ps = en(nc.semaphore("s_ps"))
        s_rt = en(nc.semaphore("s_rt"))
        s_cmp = en(nc.semaphore("s_cmp"))
        s_out = en(nc.semaphore("s_out"))
        block = en(nc.Block())

        n_banks = (out_len + 511) // 512
        bank_cols = min(512, out_len)
        bank_order = [0, 2, 1, 3][:n_banks] if n_banks == 4 else list(range(n_banks))
        rt_pos = {k: j + 1 for j, k in enumerate(bank_order)}
        s_ld = {}
        chains = []
        for ct in range(n_ct):
            for p, (xs, xe, mode) in enumerate(plan[ct]):
                s_ld[(ct, p)] = nc.alloc_semaphore(f"s_ld_{ct}_{p}")
                if mode == "acc4":
                    chains.append((ct, p))

        def overlap_last(ct_prev, xs, xe, val_map):
            vals = [
                val_map[(ct_prev, pp)]
                for pp, (ps, pe, _m) in enumerate(plan[ct_prev])
                if ps < xe and pe > xs and (ct_prev, pp) in val_map
            ]
            return max(vals) if vals else 0

        @block.gpsimd
        def _(gpsimd):
            nc.gpsimd.memset(ones[:, :], 1.0).then_inc(s_ones, 1)
            # Accumulate chains, hop-major across the chain pair: each hop
            # waits the previous hop of ITS chain (CCE RMW is unordered
            # across DMAs otherwise - measured), and the partner's hops
            # hide the completion latency.
            warred = set()
            for k in range(4):
                for (ct, p) in chains:
                    xs, xe, _mode = plan[ct][p]
                    nh = (xe - xs) // 4
                    if ct >= n_xbuf and (ct, p) not in warred:
                        gpsimd.wait_ge(
                            s_cmp, overlap_last(ct - n_xbuf, xs, xe, tmp_val)
                        )
                        warred.add((ct, p))
                    if k > 0:
                        gpsimd.wait_ge(s_ld[(ct, p)], 16 * k)
                    gpsimd.dma_start(
                        out=s4s[ct % n_xbuf][:, :, xs // 8:xs // 8 + nh // 2],
                        in_=xT[
                            ct * P:(ct + 1) * P,
                            xs + k * nh:xs + (k + 1) * nh,
                        ],
                        accum_op=(
                            mybir.AluOpType.bypass if k == 0
                            else mybir.AluOpType.add
                        ),
                    ).then_inc(s_ld[(ct, p)], 16)

        @block.sync
        def _(sync):
            # RAW piece loads on the SP HWDGE ring, as early as the two
            # xtr buffers allow.
            for i, (ct, p) in enumerate(raws):
                xs, xe, _mode = plan[ct][p]
                if i >= 4:
                    sync.wait_ge(s_cmp, pool_val[raws[i - 4]])
                sync.dma_start(
                    out=xtrs[i % 4][:, 0:(xe - xs)],
                    in_=xT[ct * P:(ct + 1) * P, xs:xe],
                ).then_inc(s_ld[(ct, p)], 16)

        @block.tensor
        def _(tensor):
            tensor.wait_ge(s_rrow, 16)
            ones_ap = rrow[:, out_len:out_len + P]
            for k in bank_order:
                nc.tensor.matmul(
                    rps[:, k * bank_cols:(k + 1) * bank_cols],
                    ones_ap,
                    rrow[:, k * bank_cols:(k + 1) * bank_cols],
                    start=True,
                    stop=True,
                ).then_inc(s_ps, 1)

        @block.vector
        def _(vector):
            cval = 0
            rt_seen = [0]

            def need_rt(pos):
                if pos > rt_seen[0]:
                    vector.wait_ge(s_rt, pos)
                    rt_seen[0] = pos

            for kind, ct, p in _ops():
                xs, xe, mode = plan[ct][p]
                n, n4, n8 = xe - xs, (xe - xs) // 4, (xe - xs) // 8
                n2 = n // 2
                xs8, xe8 = xs // 8, xe // 8
                s4t = s4s[ct % n_xbuf]
                s4e = s4t[:, 0, xs8:xe8]
                s4o = s4t[:, 1, xs8:xe8]
                cs = css[ct % 2]
                if kind == "pools":
                    # merged pairwise pools via 3D APs (contiguous inner):
                    # L1: [a|b],[c|d] = [x0|x2],[x4|x6] + [x1|x3],[x5|x7]
                    # L2: [s4e],[s4o] = [a],[c] + [b],[d]
                    xtr = xtrs[raw_idx[(ct, p)] % 4]
                    vector.wait_ge(s_ld[(ct, p)], 16)
                    xv = xtr[:, 0:n].rearrange("p (two h) -> p two h", two=2)
                    sv = s2r[:, 0:n2].rearrange("p (two h) -> p two h", two=2)
                    nc.vector.tensor_add(
                        sv, xv[:, :, 0:n4], xv[:, :, n4:n2]
                    ).then_inc(s_cmp, 1)
                    nc.vector.tensor_add(
                        s4t[:, :, xs8:xe8], sv[:, :, 0:n8], sv[:, :, n8:n4]
                    ).then_inc(s_cmp, 1)
                    cval += 2
                    assert cval == pool_val[(ct, p)]
                elif kind == "scan":
                    if mode == "acc4":
                        vector.wait_ge(s_ld[(ct, p)], 64)
                    initial = 0.0 if p == 0 else cs[:, 1, xs8 - 1:xs8]
                    nc.vector.tensor_tensor_scan(
                        cs[:, 1, xs8:xe8],
                        s4e,
                        s4o,
                        initial,
                        mybir.AluOpType.add,
                        mybir.AluOpType.add,
                    ).then_inc(s_cmp, 1)
                    cval += 1
                elif kind == "tmp":
                    # tmp[k] = cs[k] - s4o[k]  (== s4e[k] + cs[k-1])
                    nc.vector.tensor_sub(
                        cs[:, 0, xs8:xe8], cs[:, 1, xs8:xe8], s4o
                    ).then_inc(s_cmp, 1)
                    cval += 1
                    assert cval == tmp_val[(ct, p)]
                else:  # omul
                    need_rt(rt_pos[(half + xe8 - 1) // bank_cols])
                    nc.vector.tensor_mul(
                        ot[:, ct, :, xs8:xe8],
                        cs[:, :, xs8:xe8],
                        rt[:, :, xs8:xe8],
                    ).then_inc(s_cmp, 1)
                    cval += 1
                    assert cval == out_val[(ct, p)]

        @block.scalar
        def _(scalar):
            scalar.dma_start(out=rrow[:, :], in_=recip[:, :]).then_inc(s_rrow, 16)
            scalar.wait_ge(s_ones, 1)
            nc.scalar.copy(scr[:, :], ones[:, 0:1])
            for j, k in enumerate(bank_order):
                scalar.wait_ge(s_ps, j + 1)
                nc.scalar.copy(
                    rt[:, k // 2, (k % 2) * bank_cols:(k % 2 + 1) * bank_cols],
                    rps[:, k * bank_cols:(k + 1) * bank_cols],
                ).then_inc(s_rt, 1)
            n_stores = 0
            for ct in range(n_ct):
                if all(m != "acc4" for (_x, _e, m) in plan[ct]):
                    # one contiguous whole-tile store (4 KiB rows -> full
                    # packets) once the tile's last out-mul lands.
                    scalar.wait_ge(
                        s_cmp, max(out_val[(ct, p)] for p in range(len(plan[ct])))
                    )
                    scalar.dma_start(
                        out=outT[ct * P:(ct + 1) * P, :, :],
                        in_=ot[:, ct, :, :],
                    ).then_inc(s_out, 16)
                    n_stores += 1
                    continue
                for p, (xs, xe, mode) in enumerate(plan[ct]):
                    xs8, xe8 = xs // 8, xe // 8
                    scalar.wait_ge(s_cmp, out_val[(ct, p)])
                    scalar.dma_start(
                        out=outT[ct * P:(ct + 1) * P, :, xs8:xe8],
                        in_=ot[:, ct, :, xs8:xe8],
                    ).then_inc(s_out, 16)
                    n_stores += 1
            scalar.wait_ge(s_out, 16 * n_stores)

    return nc


def _recip_row(out_len):
    # [recip_even | recip_odd | ones]: even outputs 2k scale 1/(4(2k+1)),
    # odd outputs 2k+1 scale 1/(8(k+1)).
    k = np.arange(out_len // 2, dtype=np.float64)
    even = 1.0 / (4.0 * (2.0 * k + 1.0))
    odd = 1.0 / (8.0 * (k + 1.0))
    row = np.concatenate(
        [even.astype(np.float32), odd.astype(np.float32), np.ones(P, np.float32)]
    )
    return row.reshape(1, out_len + P)


def _x_perm(length):
    """Per-tile column permutation (xk = piece cols congruent k mod 8):
    raw  pieces [x0|x2|x1|x3|x4|x6|x5|x7] (merged-pool blocks),
    acc4 pieces [x0|x4|x1|x5|x2|x6|x3|x7] (hop k = [x_k | x_{k+4}])."""
    orders = {
        "acc4": (0, 4, 1, 5, 2, 6, 3, 7),
        "raw": (0, 2, 1, 3, 4, 6, 5, 7),
    }
    perms = []
    for tile in _plan(length):
        idx = []
        for xs, xe, mode in tile:
            for r in orders[mode]:
                idx.extend(range(xs + r, xe, 8))
        perms.append(np.asarray(idx))
    return perms


def prep_in_maps(x):
    b, length, d = x.shape
    xT = np.swapaxes(np.asarray(x, dtype=np.float32), 1, 2).astype(BF16)
    perms = _x_perm(length)
    n_ct = d // P
    xTp = np.empty_like(xT)
    for ct in range(n_ct):
        xTp[:, ct * P:(ct + 1) * P, :] = xT[:, ct * P:(ct + 1) * P, :][:, :, perms[ct]]
    recip = _recip_row(length // SF)
    return [{"xT": xTp[i], "recip": recip} for i in range(b)]


def post(results, b):
    outT = np.stack([np.asarray(results[i]["outT"]) for i in range(b)])
    bs, d, _two, half = outT.shape
    full = np.empty((bs, d, 2 * half), dtype=np.float32)
    full[:, :, 0::2] = outT[:, :, 0, :].astype(np.float32)
    full[:, :, 1::2] = outT[:, :, 1, :].astype(np.float32)
    return np.ascontiguousarray(np.swapaxes(full, 1, 2))


def kernel(x: np.ndarray) -> np.ndarray:
    b, length, d = x.shape
    in_maps = prep_in_maps(x)
    nc = build_bass(d=d, length=length)
    res = run_bass_kernel_spmd(nc, in_maps, core_ids=list(range(b)))
    return post(res.results, b)
